# revision 56
# baseline (speedup 1.0000x reference)
"""AttentionFreeTransformer kernel for 8 TRN2 NeuronCores.

Reference computation (B=4, T=4096, D=2048):
    qkv = rmsnorm(x) @ w_qkv.T            # [B, T, 3D]
    q, k, v = split(qkv)
    q = rmsnorm(q); k = rmsnorm(k)
    w = exp(k); kv = w * v
    y = cumsum(kv, T) / (cumsum(w, T) + 1e-6)
    out = (x, sigmoid(q) * y)

Sharding: core = 2*b + h owns batch b, sequence half h (TL=2048 tokens).
Device tensors live transposed [channel partitions, token free] so the
T-cumsum is a DVE tensor_tensor_scan along the free axis; the cross-core
carry (first-half column totals -> second-half core) is the scan's
per-partition `initial`, exchanged with one 16KB pairwise AllReduce.

Schedule (PE streams matmuls back-to-back; everything else hides under it):
  K phase   tci-outer in four j-sub-phases with resident K weights chasing
            the xT chunk DMAs (first MM ~14us).  k ssq per chunk (ACT copy
            -> DVE square -> staggered PE reduce MM); x squares spread
            through the loop; k spilled to DRAM.
  V phase   j-outer streamed weights; v=psum*inv_x (DVE), kn=k*inv_k
            (gpsimd only -- spills ride the sync queue so gpsimd never
            cascades), w=exp (ACT, accum), kv=w*v (DVE stt, accum); w/kv
            spilled.  First-half carry AllReduce fires mid-V so Q's scans
            start immediately; second at V end.
  Q phase   j-outer streamed weights; q copied (ACT) + squared (DVE) with
            two-group-staggered quad ssq MMs (col-group concurrency);
            scans + per-chunk ln/exp + y-mul interleaved one channel per
            group; y resident in SBUF for the second half of channels
            (wkp buffers are dead after K), q kept resident for the last
            two; the rest spilled and prefetched back for the tail.
  tail      inv_q chain, then per channel sigmoid(q*inv_q)*y in dedicated
            small pools (no WAR chaining), bf16 out.  HBM-bound at ~3.3us
            per channel (ql+yl reloads + out writes).

Algebraic notes:
  - rmsnorm(x)'s per-token scale inv_x factors out of the projection;
    q and k are re-rmsnormed which cancels it, so only v needs inv_x.
  - rsqrt/reciprocal via exp(-0.5*ln(.)) / exp(-ln(.)) on ACT
    (natural_log_exp table set; Rsqrt/Reciprocal ACT funcs banned).
"""

import sys

sys.path.insert(0, "/opt/trn_rl_repo")

import numpy as np
import ml_dtypes

import concourse.bass as bass
import concourse.bacc as bacc_mod
import concourse.mybir as mybir
from concourse.bass import ds, ts
from concourse.tile import TileContext

BF16 = ml_dtypes.bfloat16
F32EPS = float(np.finfo(np.float32).eps)

B, T, D = 4, 4096, 2048
NCORES = 8
TL = T // 2  # tokens per core

AF = mybir.ActivationFunctionType
ALU = mybir.AluOpType


class _Bacc(bacc_mod.Bacc):
    """Bacc whose act-table chooser maps all our funcs to one set.

    Forces Exp/Ln/Square/Copy -> natural_log_exp_and_others and
    Sigmoid -> sigmoid_and_others: 2 ACT_TABLE_LOADs total."""

    def insert_act_table_loads(self):
        from concourse.hw_specs import get_activation_tables
        from concourse.bacc import _bass_rust

        has_activation = any(
            isinstance(i, mybir.InstActivation)
            for b in self.main_func.blocks
            for i in b.instructions
        )
        if not has_activation:
            return
        ours = {AF.Exp, AF.Ln, AF.Square, AF.Copy, AF.Identity, AF.Sigmoid}
        tables = []
        for name, funcs in get_activation_tables(self.m.arch).items():
            if name == "natural_log_exp_and_others":
                tables.append((name, funcs))
            elif name == "sigmoid_and_others":
                tables.append((name, (funcs - ours) | {AF.Sigmoid}))
            else:
                tables.append((name, funcs - ours))
        _bass_rust.insert_act_table_loads(self, tables)


def build_kernel(D_=D, TL_=TL, n_cores=NCORES):
    P = 128
    CH = 512              # token chunk (psum free dim)
    ND = D_ // P          # channel subtiles per projection
    NT = TL_ // CH        # token chunks
    NDH = max(ND // 4, 1) # j-group size for the K phase sub-phases
    inv_scale = 1.0 / D_

    nc = _Bacc(target_bir_lowering=False, num_devices=n_cores)

    f32 = mybir.dt.float32
    bf16 = mybir.dt.bfloat16

    xT_h = nc.declare_dram_parameter("xT", [P, ND, TL_], bf16, isOutput=False)
    wT_h = nc.declare_dram_parameter("wT", [3 * ND, P, ND, P], bf16, isOutput=False)
    cmask_h = nc.declare_dram_parameter("cmask", [P, 1], f32, isOutput=False)
    smask_h = nc.declare_dram_parameter("smask", [P, 1], f32, isOutput=False)
    out_h = nc.declare_dram_parameter("outT", [ND, P, TL_], bf16, isOutput=True)

    ones_col_h = nc.inline_tensor(np.ones((P, 1), dtype=BF16), name="ones_col")
    ones_row_h = nc.inline_tensor(np.ones((1, P), dtype=BF16), name="ones_row")

    groups = [[i, i + 1] for i in range(0, n_cores, 2)]

    with (
        TileContext(nc) as tc,
        tc.tile_pool(name="const", bufs=1) as const,
        tc.tile_pool(name="wk", bufs=NDH + 1) as wkp,      # K weights (sub-phase)
        tc.tile_pool(name="wstream", bufs=3) as wstream,   # V/Q streamed weights
        tc.tile_pool(name="xp", bufs=1) as xp,             # xT chunks 1..NT-1
        tc.tile_pool(name="chunk", bufs=11) as chunkp,     # [P,CH] bf16 chunks
        tc.tile_pool(name="s16", bufs=12) as s16,          # [P,TL] bf16 scratch
        tc.tile_pool(name="lwc", bufs=2) as lwc,           # [P,CH] f32 ln scratch
        tc.tile_pool(name="qip", bufs=2) as qip,           # tail qi tiles
        tc.tile_pool(name="sgp", bufs=2) as sgp,           # tail sigmoid tiles
        tc.tile_pool(name="outp", bufs=2) as outp,         # tail out tiles
        tc.tile_pool(name="mmps", bufs=5, space="PSUM") as mmps,
        tc.tile_pool(name="ssqps", bufs=2, space="PSUM") as ssqps,
        tc.tile_pool(name="repps", bufs=1, space="PSUM") as repps,
        tc.tile_pool(name="spill", bufs=1, space="DRAM") as spill,
    ):
        # ---- constants / resident tiles (DMAs issued after the hot loads) ----
        ones_col = const.tile([P, 1], bf16, tag="ones_col")
        ones_row = const.tile([1, P], bf16, tag="ones_row")
        cmask = const.tile([P, 1], f32, tag="cmask")
        smask = const.tile([P, 1], f32, tag="smask")

        eps_b = const.tile([P, 1], f32, tag="eps_b")
        nc.vector.memset(eps_b[:], F32EPS)
        eps6_b = const.tile([P, 1], f32, tag="eps6_b")
        nc.vector.memset(eps6_b[:], 1e-6)

        inv_x = const.tile([P, TL_], bf16, tag="inv_x")
        inv_k = const.tile([P, TL_], bf16, tag="inv_k")
        inv_q = const.tile([P, TL_], bf16, tag="inv_q")
        # carry totals split in channel halves so the first AllReduce can fire
        # mid-V; flat layout [P, 2*n]: channel c -> cols (2c, 2c+1) = (w, kv)
        NHALF = max(ND // 2, 1)
        NB = ND - NHALF
        carryA = const.tile([P, 2 * NHALF], f32, tag="carryA")
        carry_useA = const.tile([P, 2 * NHALF], f32, tag="carry_useA")
        sndA = const.tile([P, 2 * NHALF], f32, tag="sndA")
        rcvA = const.tile([P, 2 * NHALF], f32, tag="rcvA")
        if NB:
            carryB = const.tile([P, 2 * NB], f32, tag="carryB")
            carry_useB = const.tile([P, 2 * NB], f32, tag="carry_useB")
            # snd/rcv staging reused across the two sequential exchanges
            sndB = sndA
            rcvB = rcvA

        def carry_ap(c, idx):
            if c < NHALF:
                return carryA[:, 2 * c + idx : 2 * c + idx + 1]
            return carryB[:, 2 * (c - NHALF) + idx : 2 * (c - NHALF) + idx + 1]

        def carry_use_ap(c, idx):
            if c < NHALF:
                return carry_useA[:, 2 * c + idx : 2 * c + idx + 1]
            return carry_useB[:, 2 * (c - NHALF) + idx : 2 * (c - NHALF) + idx + 1]

        # xT as chunk tiles so matmuls chase the load; chunk 0 split in two
        # half-tiles so the very first groups start on half the data.
        xT_c = {}
        if ND > 1:
            bounds = [0, ND // 2, ND]
        else:
            bounds = [0, ND]
        x0_splits = []
        for si in range(len(bounds) - 1):
            lo, hi = bounds[si], bounds[si + 1]
            t = xp.tile([P, hi - lo, CH], bf16, tag=f"xT0_{si}",
                        name=f"xT_c0_{si}")
            x0_splits.append((si, t, lo, hi))
        for tci in range(1, NT):
            xT_c[(tci, 0)] = xp.tile([P, ND, CH], bf16, tag=f"xT{tci}",
                                     name=f"xT_c{tci}")

        def x_ap(tci, do):
            if tci == 0:
                for _, t, lo, hi in x0_splits:
                    if lo <= do < hi:
                        return t[:, do - lo, :]
            return xT_c[(tci, 0)][:, do, :]

        # ---- DRAM spill arrays ----
        k_sp = spill.tile([ND, P, TL_], bf16, tag="k_sp")
        q_sp = spill.tile([ND, P, TL_], bf16, tag="q_sp")
        w_sp = spill.tile([ND, P, TL_], bf16, tag="w_sp")
        kv_sp = spill.tile([ND, P, TL_], bf16, tag="kv_sp")
        y_sp = spill.tile([ND, P, TL_], bf16, tag="y_sp")
        ccA_in = spill.tile([P, 2 * NHALF], f32, tag="ccA_in")
        ccA_out = spill.tile([P, 2 * NHALF], f32, tag="ccA_out")
        if NB:
            ccB_in = spill.tile([P, 2 * NB], f32, tag="ccB_in")
            ccB_out = spill.tile([P, 2 * NB], f32, tag="ccB_out")

        def carry_exchange(carry_t, snd_t, rcv_t, use_t, in_h, out_hh):
            nc.vector.tensor_scalar_mul(snd_t[:], carry_t[:], smask[:])
            nc.gpsimd.dma_start(out=in_h[:], in_=snd_t[:])
            nc.gpsimd.collective_compute(
                "AllReduce", ALU.add, replica_groups=groups,
                ins=[in_h[:]], outs=[out_hh[:]],
            )
            nc.sync.dma_start(out=rcv_t[:], in_=out_hh[:])
            nc.vector.tensor_scalar_mul(use_t[:], rcv_t[:], cmask[:])

        # ---- input DMAs: first K weight block + xT chunk 0 first ----
        wk_sb = {}
        wk_sb[0] = wkp.tile([P, ND, P], bf16, tag="wk", name="wk0")
        nc.scalar.dma_start(out=wk_sb[0][:], in_=wT_h[ND + 0])
        for _, t, lo, hi in x0_splits:
            nc.sync.dma_start(out=t[:], in_=xT_h[:, lo:hi, ts(0, CH)])
        for j in range(1, NDH):
            wk_sb[j] = wkp.tile([P, ND, P], bf16, tag="wk", name=f"wk{j}")
            nc.sync.dma_start(out=wk_sb[j][:], in_=wT_h[ND + j])
        for tci in range(1, NT):
            nc.sync.dma_start(out=xT_c[(tci, 0)][:], in_=xT_h[:, :, ts(tci, CH)])
        nc.sync.dma_start(out=ones_col[:], in_=ones_col_h[:])
        nc.sync.dma_start(out=ones_row[:], in_=ones_row_h[:])
        nc.sync.dma_start(out=cmask[:], in_=cmask_h[:])
        nc.sync.dma_start(out=smask[:], in_=smask_h[:])

        # ssq accumulators: one [P,CH] psum tile per projection, row 32*tci
        xssq = ssqps.tile([P, CH], f32, tag="ssq", name="xssq")
        kssq = ssqps.tile([P, CH], f32, tag="ssq", name="kssq")

        def proj_group(wsb, tci, name):
            """One accumulation group: psum[P,CH] = w_blk.T @ xT chunk."""
            pk = mmps.tile([P, CH], f32, tag="mm", name=name)
            for do in range(ND):
                nc.tensor.matmul(
                    out=pk[:],
                    lhsT=wsb[:, do, :],
                    rhs=x_ap(tci, do),
                    start=(do == 0),
                    stop=(do == ND - 1),
                )
            return pk

        def ssq_mm(ssq_tile, sq_chunk, tci, start, stop):
            # explicit tile_position: auto-derive rejects base partition 96
            nc.tensor.matmul(
                out=ssq_tile[32 * tci : 32 * tci + 1, :],
                lhsT=ones_col[:],
                rhs=sq_chunk[:],
                start=start,
                stop=stop,
                tile_position=(0, 32 * tci),
            )

        def inv_chain(ssq_tile, dest, extra_scale, rep_pool=None):
            """dest[p,t] = (ssq[t]/D + eps) ** (extra_scale) replicated."""
            rep_pool = rep_pool or repps
            row = s16.tile([1, TL_], bf16, tag="s16", name="invrow")
            for tci in range(NT):
                nc.scalar.copy(
                    out=row[:, ts(tci, CH)],
                    in_=ssq_tile[32 * tci : 32 * tci + 1, :],
                )
            for tci in range(NT):
                rep = rep_pool.tile(
                    [P, CH], f32,
                    tag="mm" if rep_pool is mmps else "rep", name="rep"
                )
                nc.tensor.matmul(
                    out=rep[:],
                    lhsT=ones_row[:],
                    rhs=row[:, ts(tci, CH)],
                    start=True,
                    stop=True,
                )
                lnc = lwc.tile([P, CH], f32, tag="lnc", name="lnc")
                nc.scalar.activation(
                    lnc[:], rep[:], AF.Ln, bias=eps_b[:], scale=inv_scale
                )
                nc.scalar.activation(
                    dest[:, ts(tci, CH)], lnc[:], AF.Exp, scale=extra_scale
                )

        # ================= K phase =================
        # two j-halves; tci-outer within each so MMs chase the xT chunks.
        # k ssq: row 32*tci accumulates over all ND j's (across both halves).
        pending_kssq = []  # staggered one group to keep PE in-order happy

        def flush_kssq(n):
            while len(pending_kssq) > n:
                pending_kssq.pop(0)()

        # x-ssq work is spread through the K loop (one do-row of DVE squares
        # every few groups, quad MMs one row behind) so the DVE queue never
        # bunches at the K->V boundary.
        pending_xmm = []
        xsq_next = [0]
        total_groups = (ND // NDH) * NT * NDH
        xw_start = NDH * NT
        xw_every = max(1, (total_groups - xw_start) // max(ND, 1))

        def emit_x_sq():
            do = xsq_next[0]
            if do >= ND:
                return
            xsq_next[0] += 1
            sq = s16.tile([P, TL_], bf16, tag="s16", name=f"xsq{do}")
            for tci in range(NT):
                nc.vector.tensor_mul(
                    out=sq[:, ts(tci, CH)],
                    in0=x_ap(tci, do), in1=x_ap(tci, do),
                )
            pending_xmm.append((do, sq))

        def flush_xmm(n):
            while len(pending_xmm) > n:
                do, sq = pending_xmm.pop(0)
                for tci in range(NT):
                    ssq_mm(xssq, sq[:, ts(tci, CH)], tci,
                           start=(do == 0), stop=(do == ND - 1))

        gidx = 0
        for jh in range(ND // NDH):
            j0 = jh * NDH
            # sub-phase 0 is tci-outer (chases the xT chunk DMAs); later
            # sub-phases are j-outer so each j's chunks finish together and
            # the k-ssq reduce MMs pack into one concurrent quad.
            if jh == 0:
                order = [(tci, jj) for tci in range(NT) for jj in range(NDH)]
            else:
                order = [(tci, jj) for jj in range(NDH) for tci in range(NT)]
            j_sqs = {}
            for tci, jj in order:
                j = j0 + jj
                if j not in wk_sb:
                    wk_sb[j] = wkp.tile([P, ND, P], bf16, tag="wk", name=f"wk{j}")
                    nc.sync.dma_start(out=wk_sb[j][:], in_=wT_h[ND + j])
                pk = proj_group(wk_sb[j], tci, f"pk{j}_{tci}")
                ksb = chunkp.tile([P, CH], bf16, tag="ch", name=f"k{j}_{tci}")
                nc.scalar.copy(out=ksb[:], in_=pk[:])
                ksq = chunkp.tile([P, CH], bf16, tag="ch", name=f"ksq{j}_{tci}")
                nc.vector.tensor_mul(out=ksq[:], in0=ksb[:], in1=ksb[:])
                nc.gpsimd.dma_start(out=k_sp[j, :, ts(tci, CH)], in_=ksb[:])
                if jh == 0:
                    pending_kssq.append(
                        (lambda kq=ksq, tc_=tci, j_=j: ssq_mm(
                            kssq, kq, tc_, start=(j_ == 0), stop=(j_ == ND - 1)))
                    )
                    flush_kssq(4)
                else:
                    j_sqs.setdefault(j, []).append(ksq)
                    if len(j_sqs[j]) == NT:
                        sq4 = j_sqs.pop(j)
                        pending_kssq.append(
                            (lambda s4=sq4, j_=j: [ssq_mm(
                                kssq, s4[tc_], tc_, start=(j_ == 0),
                                stop=(j_ == ND - 1)) for tc_ in range(NT)])
                        )
                        flush_kssq(1)
                gidx += 1
                if gidx > xw_start and (gidx - xw_start) % xw_every == 0:
                    emit_x_sq()
                    flush_xmm(1)
        while xsq_next[0] < ND:
            emit_x_sq()
            flush_xmm(1)
        flush_xmm(0)
        flush_kssq(0)

        wv_tiles = {}
        for c in range(min(2, ND)):  # prefetch first V weight blocks
            wv_tiles[c] = wstream.tile([P, ND, P], bf16, tag="wv", name=f"wv{c}")
            nc.sync.dma_start(out=wv_tiles[c][:], in_=wT_h[2 * ND + c])

        inv_chain(kssq, inv_k, -0.5)
        inv_chain(xssq, inv_x, -0.5)

        # ---- scan / y pipeline helpers ----
        scan_tiles = {}
        y_resident = {}
        qsb_keep = {}

        scan_loads = {}

        def prefetch_scan(c):
            wld = s16.tile([P, TL_], bf16, tag="s16", name=f"wld{c}")
            nc.sync.dma_start(out=wld[:], in_=w_sp[c])
            kvld = s16.tile([P, TL_], bf16, tag="s16", name=f"kvld{c}")
            nc.sync.dma_start(out=kvld[:], in_=kv_sp[c])
            scan_loads[c] = (wld, kvld)

        def emit_scan(c, pool, tg):
            """Run both cumsum scans for channel c (loads prefetched)."""
            if c not in scan_loads:
                prefetch_scan(c)
            wld, kvld = scan_loads.pop(c)
            wcum = pool.tile([P, TL_], bf16, tag=tg, name=f"wcum{c}")
            nc.vector.tensor_tensor_scan(
                out=wcum[:], data0=wld[:], data1=wld[:],
                initial=carry_use_ap(c, 0),
                op0=ALU.add, op1=ALU.bypass,
            )
            kvcum = pool.tile([P, TL_], bf16, tag=tg, name=f"kvcum{c}")
            nc.vector.tensor_tensor_scan(
                out=kvcum[:], data0=kvld[:], data1=kvld[:],
                initial=carry_use_ap(c, 1),
                op0=ALU.add, op1=ALU.bypass,
            )
            scan_tiles[c] = (wcum, kvcum)

        def emit_y(c):
            """y = kvcum * exp(-ln(wcum + 1e-6)); resident for late channels."""
            wcum, kvcum = scan_tiles.pop(c)
            rw = s16.tile([P, TL_], bf16, tag="s16", name=f"rw{c}")
            for tci in range(NT):
                lnc = lwc.tile([P, CH], f32, tag="lnc", name=f"lwy{c}_{tci}")
                nc.scalar.activation(
                    lnc[:], wcum[:, ts(tci, CH)], AF.Ln, bias=eps6_b[:]
                )
                nc.scalar.activation(
                    rw[:, ts(tci, CH)], lnc[:], AF.Exp, scale=-1.0
                )
            resident = c >= NHALF
            late = c >= ND - 3
            pool, tg = (s16, "s16") if (late or not resident) else (wkp, "wk")
            yc = pool.tile([P, TL_], bf16, tag=tg, name=f"y{c}")
            y_eng = nc.vector if late else nc.gpsimd
            y_eng.tensor_mul(out=yc[:], in0=kvcum[:], in1=rw[:])
            if resident:
                y_resident[c] = yc
            else:
                nc.gpsimd.dma_start(out=y_sp[c], in_=yc[:])

        # ================= V phase (+ first-half scans) =================
        kc_tiles = {}
        for c in range(min(2, ND)):  # prefetch k reloads
            kc_tiles[c] = s16.tile([P, TL_], bf16, tag="s16", name=f"kc{c}")
            nc.sync.dma_start(out=kc_tiles[c][:], in_=k_sp[c])

        for c in range(ND):
            if c not in wv_tiles:
                wv_tiles[c] = wstream.tile([P, ND, P], bf16, tag="wv",
                                           name=f"wv{c}")
                nc.sync.dma_start(out=wv_tiles[c][:], in_=wT_h[2 * ND + c])
            wv = wv_tiles.pop(c)
            vsb = s16.tile([P, TL_], bf16, tag="s16", name=f"v{c}")
            for tci in range(NT):
                pv = proj_group(wv, tci, f"pv{c}_{tci}")
                nc.vector.tensor_mul(
                    out=vsb[:, ts(tci, CH)], in0=pv[:], in1=inv_x[:, ts(tci, CH)]
                )
            if c + 2 < ND:
                kc_tiles[c + 2] = s16.tile([P, TL_], bf16, tag="s16",
                                           name=f"kc{c + 2}")
                nc.sync.dma_start(out=kc_tiles[c + 2][:], in_=k_sp[c + 2])
            kn = s16.tile([P, TL_], bf16, tag="s16", name=f"kn{c}")
            nc.gpsimd.tensor_mul(out=kn[:], in0=kc_tiles.pop(c)[:], in1=inv_k[:])
            wc = s16.tile([P, TL_], bf16, tag="s16", name=f"w{c}")
            nc.scalar.activation(
                wc[:], kn[:], AF.Exp, accum_out=carry_ap(c, 0)
            )
            kvc = s16.tile([P, TL_], bf16, tag="s16", name=f"kv{c}")
            nc.vector.scalar_tensor_tensor(
                out=kvc[:], in0=wc[:], scalar=1.0, in1=vsb[:],
                op0=ALU.mult, op1=ALU.mult,
                accum_out=carry_ap(c, 1),
            )
            nc.sync.dma_start(out=w_sp[c], in_=wc[:])
            nc.sync.dma_start(out=kv_sp[c], in_=kvc[:])
            # first-half carry exchange fires mid-V so Q-phase scans can
            # start the moment the Q projection begins
            if NB and c == NHALF:
                carry_exchange(carryA, sndA, rcvA, carry_useA, ccA_in, ccA_out)

        if NB:
            carry_exchange(carryB, sndB, rcvB, carry_useB, ccB_in, ccB_out)
        else:
            carry_exchange(carryA, sndA, rcvA, carry_useA, ccA_in, ccA_out)

        # ================= Q phase (+ second-half scans) =================
        qssq = ssqps.tile([P, CH], f32, tag="ssq", name="qssq")
        pending_qssq = []
        for j in range(ND):
            wq = wstream.tile([P, ND, P], bf16, tag="wq", name=f"wq{j}")
            nc.sync.dma_start(out=wq[:], in_=wT_h[0 + j])
            qsb = s16.tile([P, TL_], bf16, tag="s16", name=f"q{j}")
            sqs = []
            for tci in range(NT):
                pq = proj_group(wq, tci, f"pq{j}_{tci}")
                qsq = chunkp.tile([P, CH], bf16, tag="ch", name=f"qsq{j}_{tci}")
                if j == ND - 1:
                    # last group: square straight from psum, BEFORE the copy,
                    # so the final ssq quad (and inv_q) fires sooner
                    nc.scalar.activation(qsq[:], pq[:], AF.Square)
                    nc.scalar.copy(out=qsb[:, ts(tci, CH)], in_=pq[:])
                else:
                    nc.scalar.copy(out=qsb[:, ts(tci, CH)], in_=pq[:])
                    nc.vector.tensor_mul(
                        out=qsq[:], in0=qsb[:, ts(tci, CH)],
                        in1=qsb[:, ts(tci, CH)]
                    )
                sqs.append(qsq)
            if j >= ND - 2:
                qsb_keep[j] = qsb
            else:
                nc.gpsimd.dma_start(out=q_sp[j], in_=qsb[:])
            # quad ssq MMs (col-groups 0/32/64/96), staggered two groups so the
            # in-order PE queue never waits on the ACT->DVE square chain
            pending_qssq.append((j, sqs))
            if len(pending_qssq) > 2:
                jd, sq_ = pending_qssq.pop(0)
                for tci in range(NT):
                    ssq_mm(qssq, sq_[tci], tci,
                           start=(jd == 0), stop=(jd == ND - 1))
            # scans: channel j per slot, loads prefetched one slot ahead
            if j == 0:
                prefetch_scan(0)
            if j + 1 < ND:
                prefetch_scan(j + 1)
            emit_scan(j, s16, "s16")
            if j >= 1:
                emit_y(j - 1)
        while pending_qssq:
            jd, sq_ = pending_qssq.pop(0)
            for tci in range(NT):
                ssq_mm(qssq, sq_[tci], tci, start=(jd == 0), stop=(jd == ND - 1))
        # q reload banks: the xT chunk buffers die at the last projection MM,
        # exactly when the tail starts -- reload q into them at full HBM rate
        # so those channels are SBUF-resident with no s16 pool gating.
        BCAP = max(1, (ND * CH) // TL_)
        bank_slot = {}
        reload_cs = [c for c in range(ND - 3, -1, -1)]  # tail order, no keeps
        bi = 0
        for tci in range(1, NT):
            if bi >= len(reload_cs):
                break
            bt = xp.tile([P, BCAP, TL_], bf16, tag=f"xT{tci}",
                         name=f"qbank{tci}")
            for slot in range(BCAP):
                if bi >= len(reload_cs):
                    break
                c = reload_cs[bi]
                bi += 1
                nc.sync.dma_start(out=bt[:, slot, :], in_=q_sp[c])
                bank_slot[c] = (bt, slot)

        ybank_slot = {}
        y_reload_cs = [c for c in range(NHALF - 1, -1, -1)]
        yi = 0
        for si, _, lo, hi in x0_splits:
            cap = ((hi - lo) * CH) // TL_
            if cap < 1 or yi >= len(y_reload_cs):
                continue
            bt = xp.tile([P, cap, TL_], bf16, tag=f"xT0_{si}",
                         name=f"ybank{si}")
            for slot in range(cap):
                if yi >= len(y_reload_cs):
                    break
                c = y_reload_cs[yi]
                yi += 1
                nc.sync.dma_start(out=bt[:, slot, :], in_=y_sp[c])
                ybank_slot[c] = bt[:, slot, :]
        # three more y banks in the dead wstream (wq) buffers
        for wbi in range(max(0, min(3, len(y_reload_cs) - yi))):
            bt = wstream.tile([P, TL_], bf16, tag="wq", name=f"ybankw{wbi}")
            c = y_reload_cs[yi]
            yi += 1
            nc.sync.dma_start(out=bt[:], in_=y_sp[c])
            ybank_slot[c] = bt[:]

        emit_y(ND - 1)
        # inv_q chain fused with channel ND-1's qi/sigmoid chunks (its q and
        # y are SBUF-resident) so the ACT sigmoid pipeline starts early.
        c_last = ND - 1
        q15 = qsb_keep.pop(c_last, None)
        row_q = s16.tile([1, TL_], bf16, tag="s16", name="invrowq")
        for tci in range(NT):
            nc.scalar.copy(
                out=row_q[:, ts(tci, CH)],
                in_=qssq[32 * tci : 32 * tci + 1, :],
            )
        qi15 = qip.tile([P, TL_], bf16, tag="qi", name="qi15")
        sg15 = sgp.tile([P, TL_], bf16, tag="sg", name="sg15")
        for tci in range(NT):
            rep = mmps.tile([P, CH], f32, tag="mm", name="repq")
            nc.tensor.matmul(
                out=rep[:], lhsT=ones_row[:], rhs=row_q[:, ts(tci, CH)],
                start=True, stop=True,
            )
            lnc = lwc.tile([P, CH], f32, tag="lnc", name="lnq")
            nc.scalar.activation(
                lnc[:], rep[:], AF.Ln, bias=eps_b[:], scale=inv_scale
            )
            nc.scalar.activation(
                inv_q[:, ts(tci, CH)], lnc[:], AF.Exp, scale=-0.5
            )
            if q15 is not None:
                nc.vector.tensor_mul(
                    out=qi15[:, ts(tci, CH)],
                    in0=q15[:, ts(tci, CH)], in1=inv_q[:, ts(tci, CH)],
                )
                if tci >= 1:  # sigmoid one chunk behind: no ACT wait on DVE
                    nc.scalar.activation(
                        sg15[:, ts(tci - 1, CH)],
                        qi15[:, ts(tci - 1, CH)], AF.Sigmoid,
                    )
        if q15 is not None:
            nc.scalar.activation(
                sg15[:, ts(NT - 1, CH)], qi15[:, ts(NT - 1, CH)], AF.Sigmoid
            )
            yl15 = y_resident.pop(c_last)
            outc15 = outp.tile([P, TL_], bf16, tag="out", name="out15")
            nc.vector.tensor_mul(out=outc15[:], in0=sg15[:], in1=yl15[:])
            nc.sync.dma_start(out=out_h[c_last], in_=outc15[:])

        # ================= tail =================
        ql_tiles, yl_tiles = {}, {}

        def prefetch_tail(c):
            if c not in qsb_keep and c not in bank_slot:
                ql_tiles[c] = s16.tile([P, TL_], bf16, tag="s16", name=f"ql{c}")
                nc.sync.dma_start(out=ql_tiles[c][:], in_=q_sp[c])
            if c not in y_resident and c not in ybank_slot:
                yl_tiles[c] = s16.tile([P, TL_], bf16, tag="s16", name=f"yl{c}")
                nc.sync.dma_start(out=yl_tiles[c][:], in_=y_sp[c])

        tail_order = list(range(ND - 2, -1, -1))  # resident channels first
        for c in tail_order[: min(4, len(tail_order))]:
            prefetch_tail(c)
        for ci, c in enumerate(tail_order):
            if ci + 4 < len(tail_order):
                prefetch_tail(tail_order[ci + 4])
            out_eng = nc.sync
            if c in qsb_keep:
                qsrc_ap = qsb_keep.pop(c)[:]
            elif c in bank_slot:
                bt, slot = bank_slot.pop(c)
                qsrc_ap = bt[:, slot, :]
            else:
                qsrc_ap = ql_tiles.pop(c)[:]
            qi = qip.tile([P, TL_], bf16, tag="qi", name=f"qi{c}")
            nc.vector.tensor_mul(out=qi[:], in0=qsrc_ap, in1=inv_q[:])
            sg = sgp.tile([P, TL_], bf16, tag="sg", name=f"sg{c}")
            nc.scalar.activation(sg[:], qi[:], AF.Sigmoid)
            if c in y_resident:
                yl_ap = y_resident.pop(c)[:]
            elif c in ybank_slot:
                yl_ap = ybank_slot.pop(c)
            else:
                yl_ap = yl_tiles.pop(c)[:]
            outc = outp.tile([P, TL_], bf16, tag="out", name=f"out{c}")
            nc.vector.tensor_mul(out=outc[:], in0=sg[:], in1=yl_ap)
            out_eng.dma_start(out=out_h[c], in_=outc[:])

    nc.finalize()
    return nc


def make_in_maps(x, w_qkv, D_=D, TL_=TL, n_cores=NCORES):
    """Host-side shard + layout prep. Returns per-core input dicts."""
    P = 128
    ND = D_ // P
    E = w_qkv.shape[0]
    n_eblk = E // P
    b_count = x.shape[0]
    halves = n_cores // b_count

    # wT tiled: [e_blk, p, do, pe] with wtile[blk, p, do, e] = w_qkv[blk*128+e, do*128+p]
    wt = (
        np.ascontiguousarray(
            w_qkv.T.reshape(ND, P, n_eblk, P).transpose(2, 1, 0, 3)
        ).astype(BF16)
    )

    in_maps = []
    for core in range(n_cores):
        b, h = divmod(core, halves)
        shard = x[b, h * TL_ : (h + 1) * TL_, :]  # [TL, D]
        xt = np.ascontiguousarray(
            shard.T.reshape(ND, P, TL_).transpose(1, 0, 2)
        ).astype(BF16)
        odd = float(h % 2 == 1)
        in_maps.append(
            {
                "xT": xt,
                "wT": wt,
                "cmask": np.full((P, 1), odd, dtype=np.float32),
                "smask": np.full((P, 1), 1.0 - odd, dtype=np.float32),
            }
        )
    return in_maps


def assemble_output(results, x, D_=D, TL_=TL, n_cores=NCORES):
    b_count = x.shape[0]
    halves = n_cores // b_count
    out2 = np.empty((b_count, halves * TL_, D_), dtype=np.float32)
    for core in range(n_cores):
        b, h = divmod(core, halves)
        outT = np.asarray(results[core]["outT"]).astype(np.float32).reshape(D_, TL_)
        out2[b, h * TL_ : (h + 1) * TL_, :] = outT.T
    return out2


_CACHED_NC = None


def kernel(x, w_qkv):
    global _CACHED_NC
    from concourse.bass_utils import run_bass_kernel_spmd

    x = np.asarray(x, dtype=np.float32)
    w_qkv = np.asarray(w_qkv, dtype=np.float32)

    if _CACHED_NC is None:
        _CACHED_NC = build_kernel()
    in_maps = make_in_maps(x, w_qkv)
    res = run_bass_kernel_spmd(_CACHED_NC, in_maps, core_ids=list(range(NCORES)))
    out2 = assemble_output(res.results, x)
    return (x, out2)


# revision 57
# speedup vs baseline: 1.0059x; 1.0059x over previous
"""AttentionFreeTransformer kernel for 8 TRN2 NeuronCores.

Reference computation (B=4, T=4096, D=2048):
    qkv = rmsnorm(x) @ w_qkv.T            # [B, T, 3D]
    q, k, v = split(qkv)
    q = rmsnorm(q); k = rmsnorm(k)
    w = exp(k); kv = w * v
    y = cumsum(kv, T) / (cumsum(w, T) + 1e-6)
    out = (x, sigmoid(q) * y)

Sharding: core = 2*b + h owns batch b, sequence half h (TL=2048 tokens).
Device tensors live transposed [channel partitions, token free] so the
T-cumsum is a DVE tensor_tensor_scan along the free axis; the cross-core
carry (first-half column totals -> second-half core) is the scan's
per-partition `initial`, exchanged with one 16KB pairwise AllReduce.

Schedule (PE streams matmuls back-to-back; everything else hides under it):
  K phase   tci-outer in four j-sub-phases with resident K weights chasing
            the xT chunk DMAs (first MM ~14us).  k ssq per chunk (ACT copy
            -> DVE square -> staggered PE reduce MM); x squares spread
            through the loop; k spilled to DRAM.
  V phase   j-outer streamed weights; v=psum*inv_x (DVE), kn=k*inv_k
            (gpsimd only -- spills ride the sync queue so gpsimd never
            cascades), w=exp (ACT, accum), kv=w*v (DVE stt, accum); w/kv
            spilled.  First-half carry AllReduce fires mid-V so Q's scans
            start immediately; second at V end.
  Q phase   j-outer streamed weights; q copied (ACT) + squared (DVE) with
            two-group-staggered quad ssq MMs (col-group concurrency);
            scans + per-chunk ln/exp + y-mul interleaved one channel per
            group; y resident in SBUF for the second half of channels
            (wkp buffers are dead after K), q kept resident for the last
            two; the rest spilled and prefetched back for the tail.
  tail      inv_q chain, then per channel sigmoid(q*inv_q)*y in dedicated
            small pools (no WAR chaining), bf16 out.  HBM-bound at ~3.3us
            per channel (ql+yl reloads + out writes).

Algebraic notes:
  - rmsnorm(x)'s per-token scale inv_x factors out of the projection;
    q and k are re-rmsnormed which cancels it, so only v needs inv_x.
  - rsqrt/reciprocal via exp(-0.5*ln(.)) / exp(-ln(.)) on ACT
    (natural_log_exp table set; Rsqrt/Reciprocal ACT funcs banned).
"""

import sys

sys.path.insert(0, "/opt/trn_rl_repo")

import numpy as np
import ml_dtypes

import concourse.bass as bass
import concourse.bacc as bacc_mod
import concourse.mybir as mybir
from concourse.bass import ds, ts
from concourse.tile import TileContext

BF16 = ml_dtypes.bfloat16
F32EPS = float(np.finfo(np.float32).eps)

B, T, D = 4, 4096, 2048
NCORES = 8
TL = T // 2  # tokens per core

AF = mybir.ActivationFunctionType
ALU = mybir.AluOpType


class _Bacc(bacc_mod.Bacc):
    """Bacc whose act-table chooser maps all our funcs to one set.

    Forces Exp/Ln/Square/Copy -> natural_log_exp_and_others and
    Sigmoid -> sigmoid_and_others: 2 ACT_TABLE_LOADs total."""

    def insert_act_table_loads(self):
        from concourse.hw_specs import get_activation_tables
        from concourse.bacc import _bass_rust

        has_activation = any(
            isinstance(i, mybir.InstActivation)
            for b in self.main_func.blocks
            for i in b.instructions
        )
        if not has_activation:
            return
        ours = {AF.Exp, AF.Ln, AF.Square, AF.Copy, AF.Identity, AF.Sigmoid}
        tables = []
        for name, funcs in get_activation_tables(self.m.arch).items():
            if name == "natural_log_exp_and_others":
                tables.append((name, funcs))
            elif name == "sigmoid_and_others":
                tables.append((name, (funcs - ours) | {AF.Sigmoid}))
            else:
                tables.append((name, funcs - ours))
        _bass_rust.insert_act_table_loads(self, tables)


def build_kernel(D_=D, TL_=TL, n_cores=NCORES):
    P = 128
    CH = 512              # token chunk (psum free dim)
    ND = D_ // P          # channel subtiles per projection
    NT = TL_ // CH        # token chunks
    NDH = max(ND // 4, 1) # j-group size for the K phase sub-phases
    inv_scale = 1.0 / D_

    nc = _Bacc(target_bir_lowering=False, num_devices=n_cores)

    f32 = mybir.dt.float32
    bf16 = mybir.dt.bfloat16

    xT_h = nc.declare_dram_parameter("xT", [P, ND, TL_], bf16, isOutput=False)
    wT_h = nc.declare_dram_parameter("wT", [3 * ND, P, ND, P], bf16, isOutput=False)
    cmask_h = nc.declare_dram_parameter("cmask", [P, 1], f32, isOutput=False)
    smask_h = nc.declare_dram_parameter("smask", [P, 1], f32, isOutput=False)
    out_h = nc.declare_dram_parameter("outT", [ND, P, TL_], bf16, isOutput=True)

    ones_col_h = nc.inline_tensor(np.ones((P, 1), dtype=BF16), name="ones_col")
    ones_row_h = nc.inline_tensor(np.ones((1, P), dtype=BF16), name="ones_row")

    groups = [[i, i + 1] for i in range(0, n_cores, 2)]

    with (
        TileContext(nc) as tc,
        tc.tile_pool(name="const", bufs=1) as const,
        tc.tile_pool(name="wk", bufs=NDH + 1) as wkp,      # K weights (sub-phase)
        tc.tile_pool(name="wstream", bufs=3) as wstream,   # V/Q streamed weights
        tc.tile_pool(name="xp", bufs=1) as xp,             # xT chunks 1..NT-1
        tc.tile_pool(name="chunk", bufs=11) as chunkp,     # [P,CH] bf16 chunks
        tc.tile_pool(name="s16", bufs=12) as s16,          # [P,TL] bf16 scratch
        tc.tile_pool(name="lwc", bufs=2) as lwc,           # [P,CH] f32 ln scratch
        tc.tile_pool(name="qip", bufs=2) as qip,           # tail qi tiles
        tc.tile_pool(name="sgp", bufs=2) as sgp,           # tail sigmoid tiles
        tc.tile_pool(name="outp", bufs=2) as outp,         # tail out tiles
        tc.tile_pool(name="mmps", bufs=5, space="PSUM") as mmps,
        tc.tile_pool(name="ssqps", bufs=2, space="PSUM") as ssqps,
        tc.tile_pool(name="repps", bufs=1, space="PSUM") as repps,
        tc.tile_pool(name="spill", bufs=1, space="DRAM") as spill,
    ):
        # ---- constants / resident tiles (DMAs issued after the hot loads) ----
        ones_col = const.tile([P, 1], bf16, tag="ones_col")
        ones_row = const.tile([1, P], bf16, tag="ones_row")
        cmask = const.tile([P, 1], f32, tag="cmask")
        smask = const.tile([P, 1], f32, tag="smask")

        eps_b = const.tile([P, 1], f32, tag="eps_b")
        nc.vector.memset(eps_b[:], F32EPS)
        eps6_b = const.tile([P, 1], f32, tag="eps6_b")
        nc.vector.memset(eps6_b[:], 1e-6)

        inv_x = const.tile([P, TL_], bf16, tag="inv_x")
        inv_k = const.tile([P, TL_], bf16, tag="inv_k")
        inv_q = const.tile([P, TL_], bf16, tag="inv_q")
        # carry totals split in channel halves so the first AllReduce can fire
        # mid-V; flat layout [P, 2*n]: channel c -> cols (2c, 2c+1) = (w, kv)
        NHALF = max(ND // 2, 1)
        NB = ND - NHALF
        carryA = const.tile([P, 2 * NHALF], f32, tag="carryA")
        carry_useA = const.tile([P, 2 * NHALF], f32, tag="carry_useA")
        sndA = const.tile([P, 2 * NHALF], f32, tag="sndA")
        rcvA = const.tile([P, 2 * NHALF], f32, tag="rcvA")
        if NB:
            carryB = const.tile([P, 2 * NB], f32, tag="carryB")
            carry_useB = const.tile([P, 2 * NB], f32, tag="carry_useB")
            # snd/rcv staging reused across the two sequential exchanges
            sndB = sndA
            rcvB = rcvA

        def carry_ap(c, idx):
            if c < NHALF:
                return carryA[:, 2 * c + idx : 2 * c + idx + 1]
            return carryB[:, 2 * (c - NHALF) + idx : 2 * (c - NHALF) + idx + 1]

        def carry_use_ap(c, idx):
            if c < NHALF:
                return carry_useA[:, 2 * c + idx : 2 * c + idx + 1]
            return carry_useB[:, 2 * (c - NHALF) + idx : 2 * (c - NHALF) + idx + 1]

        # xT as chunk tiles so matmuls chase the load; chunk 0 split in two
        # half-tiles so the very first groups start on half the data.
        xT_c = {}
        if ND > 1:
            bounds = [0, ND // 2, ND]
        else:
            bounds = [0, ND]
        x0_splits = []
        for si in range(len(bounds) - 1):
            lo, hi = bounds[si], bounds[si + 1]
            t = xp.tile([P, hi - lo, CH], bf16, tag=f"xT0_{si}",
                        name=f"xT_c0_{si}")
            x0_splits.append((si, t, lo, hi))
        for tci in range(1, NT):
            xT_c[(tci, 0)] = xp.tile([P, ND, CH], bf16, tag=f"xT{tci}",
                                     name=f"xT_c{tci}")

        def x_ap(tci, do):
            if tci == 0:
                for _, t, lo, hi in x0_splits:
                    if lo <= do < hi:
                        return t[:, do - lo, :]
            return xT_c[(tci, 0)][:, do, :]

        # ---- DRAM spill arrays ----
        k_sp = spill.tile([ND, P, TL_], bf16, tag="k_sp")
        q_sp = spill.tile([ND, P, TL_], bf16, tag="q_sp")
        w_sp = spill.tile([ND, P, TL_], bf16, tag="w_sp")
        kv_sp = spill.tile([ND, P, TL_], bf16, tag="kv_sp")
        y_sp = spill.tile([ND, P, TL_], bf16, tag="y_sp")
        ccA_in = spill.tile([P, 2 * NHALF], f32, tag="ccA_in")
        ccA_out = spill.tile([P, 2 * NHALF], f32, tag="ccA_out")
        if NB:
            ccB_in = spill.tile([P, 2 * NB], f32, tag="ccB_in")
            ccB_out = spill.tile([P, 2 * NB], f32, tag="ccB_out")

        def carry_exchange(carry_t, snd_t, rcv_t, use_t, in_h, out_hh):
            nc.vector.tensor_scalar_mul(snd_t[:], carry_t[:], smask[:])
            nc.gpsimd.dma_start(out=in_h[:], in_=snd_t[:])
            nc.gpsimd.collective_compute(
                "AllReduce", ALU.add, replica_groups=groups,
                ins=[in_h[:]], outs=[out_hh[:]],
            )
            nc.sync.dma_start(out=rcv_t[:], in_=out_hh[:])
            nc.vector.tensor_scalar_mul(use_t[:], rcv_t[:], cmask[:])

        # ---- input DMAs: first K weight block + xT chunk 0 first ----
        wk_sb = {}
        wk_sb[0] = wkp.tile([P, ND, P], bf16, tag="wk", name="wk0")
        nc.scalar.dma_start(out=wk_sb[0][:], in_=wT_h[ND + 0])
        for _, t, lo, hi in x0_splits:
            nc.sync.dma_start(out=t[:], in_=xT_h[:, lo:hi, ts(0, CH)])
        for j in range(1, NDH):
            wk_sb[j] = wkp.tile([P, ND, P], bf16, tag="wk", name=f"wk{j}")
            nc.sync.dma_start(out=wk_sb[j][:], in_=wT_h[ND + j])
        for tci in range(1, NT):
            nc.sync.dma_start(out=xT_c[(tci, 0)][:], in_=xT_h[:, :, ts(tci, CH)])
        nc.sync.dma_start(out=ones_col[:], in_=ones_col_h[:])
        nc.sync.dma_start(out=ones_row[:], in_=ones_row_h[:])
        nc.sync.dma_start(out=cmask[:], in_=cmask_h[:])
        nc.sync.dma_start(out=smask[:], in_=smask_h[:])

        # ssq accumulators: one [P,CH] psum tile per projection, row 32*tci
        xssq = ssqps.tile([P, CH], f32, tag="ssq", name="xssq")
        kssq = ssqps.tile([P, CH], f32, tag="ssq", name="kssq")

        def proj_group(wsb, tci, name):
            """One accumulation group: psum[P,CH] = w_blk.T @ xT chunk."""
            pk = mmps.tile([P, CH], f32, tag="mm", name=name)
            for do in range(ND):
                nc.tensor.matmul(
                    out=pk[:],
                    lhsT=wsb[:, do, :],
                    rhs=x_ap(tci, do),
                    start=(do == 0),
                    stop=(do == ND - 1),
                )
            return pk

        def ssq_mm(ssq_tile, sq_chunk, tci, start, stop):
            # explicit tile_position: auto-derive rejects base partition 96
            nc.tensor.matmul(
                out=ssq_tile[32 * tci : 32 * tci + 1, :],
                lhsT=ones_col[:],
                rhs=sq_chunk[:],
                start=start,
                stop=stop,
                tile_position=(0, 32 * tci),
            )

        def inv_chain(ssq_tile, dest, extra_scale, rep_pool=None):
            """dest[p,t] = (ssq[t]/D + eps) ** (extra_scale) replicated."""
            rep_pool = rep_pool or repps
            row = s16.tile([1, TL_], bf16, tag="s16", name="invrow")
            for tci in range(NT):
                nc.scalar.copy(
                    out=row[:, ts(tci, CH)],
                    in_=ssq_tile[32 * tci : 32 * tci + 1, :],
                )
            for tci in range(NT):
                rep = rep_pool.tile(
                    [P, CH], f32,
                    tag="mm" if rep_pool is mmps else "rep", name="rep"
                )
                nc.tensor.matmul(
                    out=rep[:],
                    lhsT=ones_row[:],
                    rhs=row[:, ts(tci, CH)],
                    start=True,
                    stop=True,
                )
                lnc = lwc.tile([P, CH], f32, tag="lnc", name="lnc")
                nc.scalar.activation(
                    lnc[:], rep[:], AF.Ln, bias=eps_b[:], scale=inv_scale
                )
                nc.scalar.activation(
                    dest[:, ts(tci, CH)], lnc[:], AF.Exp, scale=extra_scale
                )

        # ================= K phase =================
        # two j-halves; tci-outer within each so MMs chase the xT chunks.
        # k ssq: row 32*tci accumulates over all ND j's (across both halves).
        pending_kssq = []  # staggered one group to keep PE in-order happy

        def flush_kssq(n):
            while len(pending_kssq) > n:
                pending_kssq.pop(0)()

        # x-ssq work is spread through the K loop (one do-row of DVE squares
        # every few groups, quad MMs one row behind) so the DVE queue never
        # bunches at the K->V boundary.
        pending_xmm = []
        xsq_next = [0]
        total_groups = (ND // NDH) * NT * NDH
        xw_start = NDH * NT
        xw_every = max(1, (total_groups - xw_start) // max(ND, 1))

        def emit_x_sq():
            do = xsq_next[0]
            if do >= ND:
                return
            xsq_next[0] += 1
            sq = s16.tile([P, TL_], bf16, tag="s16", name=f"xsq{do}")
            for tci in range(NT):
                nc.vector.tensor_mul(
                    out=sq[:, ts(tci, CH)],
                    in0=x_ap(tci, do), in1=x_ap(tci, do),
                )
            pending_xmm.append((do, sq))

        def flush_xmm(n):
            while len(pending_xmm) > n:
                do, sq = pending_xmm.pop(0)
                for tci in range(NT):
                    ssq_mm(xssq, sq[:, ts(tci, CH)], tci,
                           start=(do == 0), stop=(do == ND - 1))

        gidx = 0
        for jh in range(ND // NDH):
            j0 = jh * NDH
            # sub-phase 0 is tci-outer (chases the xT chunk DMAs); later
            # sub-phases are j-outer so each j's chunks finish together and
            # the k-ssq reduce MMs pack into one concurrent quad.
            if jh == 0:
                order = [(tci, jj) for tci in range(NT) for jj in range(NDH)]
            else:
                order = [(tci, jj) for jj in range(NDH) for tci in range(NT)]
            j_sqs = {}
            for tci, jj in order:
                j = j0 + jj
                if j not in wk_sb:
                    wk_sb[j] = wkp.tile([P, ND, P], bf16, tag="wk", name=f"wk{j}")
                    nc.sync.dma_start(out=wk_sb[j][:], in_=wT_h[ND + j])
                pk = proj_group(wk_sb[j], tci, f"pk{j}_{tci}")
                ksb = chunkp.tile([P, CH], bf16, tag="ch", name=f"k{j}_{tci}")
                nc.scalar.copy(out=ksb[:], in_=pk[:])
                ksq = chunkp.tile([P, CH], bf16, tag="ch", name=f"ksq{j}_{tci}")
                nc.vector.tensor_mul(out=ksq[:], in0=ksb[:], in1=ksb[:])
                nc.gpsimd.dma_start(out=k_sp[j, :, ts(tci, CH)], in_=ksb[:])
                if jh == 0:
                    pending_kssq.append(
                        (lambda kq=ksq, tc_=tci, j_=j: ssq_mm(
                            kssq, kq, tc_, start=(j_ == 0), stop=(j_ == ND - 1)))
                    )
                    flush_kssq(4)
                else:
                    j_sqs.setdefault(j, []).append(ksq)
                    if len(j_sqs[j]) == NT:
                        sq4 = j_sqs.pop(j)
                        pending_kssq.append(
                            (lambda s4=sq4, j_=j: [ssq_mm(
                                kssq, s4[tc_], tc_, start=(j_ == 0),
                                stop=(j_ == ND - 1)) for tc_ in range(NT)])
                        )
                        flush_kssq(1)
                gidx += 1
                if gidx > xw_start and (gidx - xw_start) % xw_every == 0:
                    emit_x_sq()
                    flush_xmm(1)
        while xsq_next[0] < ND:
            emit_x_sq()
            flush_xmm(1)
        flush_xmm(0)
        flush_kssq(0)

        wv_tiles = {}
        for c in range(min(2, ND)):  # prefetch first V weight blocks
            wv_tiles[c] = wstream.tile([P, ND, P], bf16, tag="wv", name=f"wv{c}")
            nc.sync.dma_start(out=wv_tiles[c][:], in_=wT_h[2 * ND + c])

        inv_chain(kssq, inv_k, -0.5)
        inv_chain(xssq, inv_x, -0.5)

        # ---- scan / y pipeline helpers ----
        scan_tiles = {}
        y_resident = {}
        qsb_keep = {}

        scan_loads = {}

        def prefetch_scan(c):
            wld = s16.tile([P, TL_], bf16, tag="s16", name=f"wld{c}")
            nc.sync.dma_start(out=wld[:], in_=w_sp[c])
            kvld = s16.tile([P, TL_], bf16, tag="s16", name=f"kvld{c}")
            nc.sync.dma_start(out=kvld[:], in_=kv_sp[c])
            scan_loads[c] = (wld, kvld)

        def emit_scan(c, pool, tg):
            """Run both cumsum scans for channel c (loads prefetched)."""
            if c not in scan_loads:
                prefetch_scan(c)
            wld, kvld = scan_loads.pop(c)
            wcum = pool.tile([P, TL_], bf16, tag=tg, name=f"wcum{c}")
            nc.vector.tensor_tensor_scan(
                out=wcum[:], data0=wld[:], data1=wld[:],
                initial=carry_use_ap(c, 0),
                op0=ALU.add, op1=ALU.bypass,
            )
            kvcum = pool.tile([P, TL_], bf16, tag=tg, name=f"kvcum{c}")
            nc.vector.tensor_tensor_scan(
                out=kvcum[:], data0=kvld[:], data1=kvld[:],
                initial=carry_use_ap(c, 1),
                op0=ALU.add, op1=ALU.bypass,
            )
            scan_tiles[c] = (wcum, kvcum)

        def emit_y(c):
            """y = kvcum * exp(-ln(wcum + 1e-6)); resident for late channels."""
            wcum, kvcum = scan_tiles.pop(c)
            rw = s16.tile([P, TL_], bf16, tag="s16", name=f"rw{c}")
            for tci in range(NT):
                lnc = lwc.tile([P, CH], f32, tag="lnc", name=f"lwy{c}_{tci}")
                nc.scalar.activation(
                    lnc[:], wcum[:, ts(tci, CH)], AF.Ln, bias=eps6_b[:]
                )
                nc.scalar.activation(
                    rw[:, ts(tci, CH)], lnc[:], AF.Exp, scale=-1.0
                )
            resident = c >= NHALF
            late = c >= ND - 3
            pool, tg = (s16, "s16") if (late or not resident) else (wkp, "wk")
            yc = pool.tile([P, TL_], bf16, tag=tg, name=f"y{c}")
            y_eng = nc.vector if late else nc.gpsimd
            y_eng.tensor_mul(out=yc[:], in0=kvcum[:], in1=rw[:])
            if resident:
                y_resident[c] = yc
            else:
                nc.gpsimd.dma_start(out=y_sp[c], in_=yc[:])

        # ================= V phase (+ first-half scans) =================
        kc_tiles = {}
        for c in range(min(2, ND)):  # prefetch k reloads
            kc_tiles[c] = s16.tile([P, TL_], bf16, tag="s16", name=f"kc{c}")
            nc.sync.dma_start(out=kc_tiles[c][:], in_=k_sp[c])

        for c in range(ND):
            if c not in wv_tiles:
                wv_tiles[c] = wstream.tile([P, ND, P], bf16, tag="wv",
                                           name=f"wv{c}")
                nc.sync.dma_start(out=wv_tiles[c][:], in_=wT_h[2 * ND + c])
            wv = wv_tiles.pop(c)
            vsb = s16.tile([P, TL_], bf16, tag="s16", name=f"v{c}")
            for tci in range(NT):
                pv = proj_group(wv, tci, f"pv{c}_{tci}")
                nc.vector.tensor_mul(
                    out=vsb[:, ts(tci, CH)], in0=pv[:], in1=inv_x[:, ts(tci, CH)]
                )
            if c + 2 < ND:
                kc_tiles[c + 2] = s16.tile([P, TL_], bf16, tag="s16",
                                           name=f"kc{c + 2}")
                nc.sync.dma_start(out=kc_tiles[c + 2][:], in_=k_sp[c + 2])
            kn = s16.tile([P, TL_], bf16, tag="s16", name=f"kn{c}")
            nc.gpsimd.tensor_mul(out=kn[:], in0=kc_tiles.pop(c)[:], in1=inv_k[:])
            wc = s16.tile([P, TL_], bf16, tag="s16", name=f"w{c}")
            nc.scalar.activation(
                wc[:], kn[:], AF.Exp, accum_out=carry_ap(c, 0)
            )
            kvc = s16.tile([P, TL_], bf16, tag="s16", name=f"kv{c}")
            nc.vector.scalar_tensor_tensor(
                out=kvc[:], in0=wc[:], scalar=1.0, in1=vsb[:],
                op0=ALU.mult, op1=ALU.mult,
                accum_out=carry_ap(c, 1),
            )
            nc.sync.dma_start(out=w_sp[c], in_=wc[:])
            nc.sync.dma_start(out=kv_sp[c], in_=kvc[:])
            # first-half carry exchange fires mid-V so Q-phase scans can
            # start the moment the Q projection begins
            if NB and c == NHALF:
                carry_exchange(carryA, sndA, rcvA, carry_useA, ccA_in, ccA_out)

        if NB:
            carry_exchange(carryB, sndB, rcvB, carry_useB, ccB_in, ccB_out)
        else:
            carry_exchange(carryA, sndA, rcvA, carry_useA, ccA_in, ccA_out)

        # ================= Q phase (+ second-half scans) =================
        qssq = ssqps.tile([P, CH], f32, tag="ssq", name="qssq")
        pending_qssq = []
        for j in range(ND):
            wq = wstream.tile([P, ND, P], bf16, tag="wq", name=f"wq{j}")
            nc.sync.dma_start(out=wq[:], in_=wT_h[0 + j])
            qsb = s16.tile([P, TL_], bf16, tag="s16", name=f"q{j}")
            sqs = []
            for tci in range(NT):
                pq = proj_group(wq, tci, f"pq{j}_{tci}")
                qsq = chunkp.tile([P, CH], bf16, tag="ch", name=f"qsq{j}_{tci}")
                if j == ND - 1:
                    # last group: square straight from psum, BEFORE the copy,
                    # so the final ssq quad (and inv_q) fires sooner
                    nc.scalar.activation(qsq[:], pq[:], AF.Square)
                    nc.scalar.copy(out=qsb[:, ts(tci, CH)], in_=pq[:])
                else:
                    nc.scalar.copy(out=qsb[:, ts(tci, CH)], in_=pq[:])
                    nc.vector.tensor_mul(
                        out=qsq[:], in0=qsb[:, ts(tci, CH)],
                        in1=qsb[:, ts(tci, CH)]
                    )
                sqs.append(qsq)
            if j >= ND - 2:
                qsb_keep[j] = qsb
            else:
                nc.gpsimd.dma_start(out=q_sp[j], in_=qsb[:])
            # quad ssq MMs (col-groups 0/32/64/96), staggered two groups so the
            # in-order PE queue never waits on the ACT->DVE square chain
            pending_qssq.append((j, sqs))
            if len(pending_qssq) > 2:
                jd, sq_ = pending_qssq.pop(0)
                for tci in range(NT):
                    ssq_mm(qssq, sq_[tci], tci,
                           start=(jd == 0), stop=(jd == ND - 1))
            # scans: channel j per slot, loads prefetched one slot ahead
            if j == 0:
                prefetch_scan(0)
            if j + 1 < ND:
                prefetch_scan(j + 1)
            emit_scan(j, s16, "s16")
            if j >= 1:
                emit_y(j - 1)
        while pending_qssq:
            jd, sq_ = pending_qssq.pop(0)
            for tci in range(NT):
                ssq_mm(qssq, sq_[tci], tci, start=(jd == 0), stop=(jd == ND - 1))
        # q reload banks: the xT chunk buffers die at the last projection MM,
        # exactly when the tail starts -- reload q into them at full HBM rate
        # so those channels are SBUF-resident with no s16 pool gating.
        BCAP = max(1, (ND * CH) // TL_)
        bank_slot = {}
        reload_cs = [c for c in range(ND - 3, -1, -1)]  # tail order, no keeps
        bi = 0
        for tci in range(1, NT):
            if bi >= len(reload_cs):
                break
            bt = xp.tile([P, BCAP, TL_], bf16, tag=f"xT{tci}",
                         name=f"qbank{tci}")
            for slot in range(BCAP):
                if bi >= len(reload_cs):
                    break
                c = reload_cs[bi]
                bi += 1
                nc.sync.dma_start(out=bt[:, slot, :], in_=q_sp[c])
                bank_slot[c] = (bt, slot)

        ybank_slot = {}
        y_reload_cs = [c for c in range(NHALF - 1, -1, -1)]
        yi = 0
        for si, _, lo, hi in x0_splits:
            cap = ((hi - lo) * CH) // TL_
            if cap < 1 or yi >= len(y_reload_cs):
                continue
            bt = xp.tile([P, cap, TL_], bf16, tag=f"xT0_{si}",
                         name=f"ybank{si}")
            for slot in range(cap):
                if yi >= len(y_reload_cs):
                    break
                c = y_reload_cs[yi]
                yi += 1
                nc.sync.dma_start(out=bt[:, slot, :], in_=y_sp[c])
                ybank_slot[c] = bt[:, slot, :]
        # three more y banks in the dead wstream (wq) buffers
        for wbi in range(max(0, min(3, len(y_reload_cs) - yi))):
            bt = wstream.tile([P, TL_], bf16, tag="wq", name=f"ybankw{wbi}")
            c = y_reload_cs[yi]
            yi += 1
            nc.sync.dma_start(out=bt[:], in_=y_sp[c])
            ybank_slot[c] = bt[:]

        inv_chain(qssq, inv_q, -0.5, rep_pool=mmps)
        emit_y(ND - 1)

        # ================= tail =================
        ql_tiles, yl_tiles = {}, {}

        def prefetch_tail(c):
            if c not in qsb_keep and c not in bank_slot:
                ql_tiles[c] = s16.tile([P, TL_], bf16, tag="s16", name=f"ql{c}")
                nc.sync.dma_start(out=ql_tiles[c][:], in_=q_sp[c])
            if c not in y_resident and c not in ybank_slot:
                yl_tiles[c] = s16.tile([P, TL_], bf16, tag="s16", name=f"yl{c}")
                nc.sync.dma_start(out=yl_tiles[c][:], in_=y_sp[c])

        tail_order = list(range(ND - 1, -1, -1))  # resident channels first
        for c in tail_order[: min(4, ND)]:
            prefetch_tail(c)
        for ci, c in enumerate(tail_order):
            if ci + 4 < ND:
                prefetch_tail(tail_order[ci + 4])
            out_eng = nc.sync
            if c in qsb_keep:
                qsrc_ap = qsb_keep.pop(c)[:]
            elif c in bank_slot:
                bt, slot = bank_slot.pop(c)
                qsrc_ap = bt[:, slot, :]
            else:
                qsrc_ap = ql_tiles.pop(c)[:]
            qi = qip.tile([P, TL_], bf16, tag="qi", name=f"qi{c}")
            nc.vector.tensor_mul(out=qi[:], in0=qsrc_ap, in1=inv_q[:])
            sg = sgp.tile([P, TL_], bf16, tag="sg", name=f"sg{c}")
            nc.scalar.activation(sg[:], qi[:], AF.Sigmoid)
            if c in y_resident:
                yl_ap = y_resident.pop(c)[:]
            elif c in ybank_slot:
                yl_ap = ybank_slot.pop(c)
            else:
                yl_ap = yl_tiles.pop(c)[:]
            outc = outp.tile([P, TL_], bf16, tag="out", name=f"out{c}")
            nc.vector.tensor_mul(out=outc[:], in0=sg[:], in1=yl_ap)
            out_eng.dma_start(out=out_h[c], in_=outc[:])

    nc.finalize()
    return nc


def make_in_maps(x, w_qkv, D_=D, TL_=TL, n_cores=NCORES):
    """Host-side shard + layout prep. Returns per-core input dicts."""
    P = 128
    ND = D_ // P
    E = w_qkv.shape[0]
    n_eblk = E // P
    b_count = x.shape[0]
    halves = n_cores // b_count

    # wT tiled: [e_blk, p, do, pe] with wtile[blk, p, do, e] = w_qkv[blk*128+e, do*128+p]
    wt = (
        np.ascontiguousarray(
            w_qkv.T.reshape(ND, P, n_eblk, P).transpose(2, 1, 0, 3)
        ).astype(BF16)
    )

    in_maps = []
    for core in range(n_cores):
        b, h = divmod(core, halves)
        shard = x[b, h * TL_ : (h + 1) * TL_, :]  # [TL, D]
        xt = np.ascontiguousarray(
            shard.T.reshape(ND, P, TL_).transpose(1, 0, 2)
        ).astype(BF16)
        odd = float(h % 2 == 1)
        in_maps.append(
            {
                "xT": xt,
                "wT": wt,
                "cmask": np.full((P, 1), odd, dtype=np.float32),
                "smask": np.full((P, 1), 1.0 - odd, dtype=np.float32),
            }
        )
    return in_maps


def assemble_output(results, x, D_=D, TL_=TL, n_cores=NCORES):
    b_count = x.shape[0]
    halves = n_cores // b_count
    out2 = np.empty((b_count, halves * TL_, D_), dtype=np.float32)
    for core in range(n_cores):
        b, h = divmod(core, halves)
        outT = np.asarray(results[core]["outT"]).astype(np.float32).reshape(D_, TL_)
        out2[b, h * TL_ : (h + 1) * TL_, :] = outT.T
    return out2


_CACHED_NC = None


def kernel(x, w_qkv):
    global _CACHED_NC
    from concourse.bass_utils import run_bass_kernel_spmd

    x = np.asarray(x, dtype=np.float32)
    w_qkv = np.asarray(w_qkv, dtype=np.float32)

    if _CACHED_NC is None:
        _CACHED_NC = build_kernel()
    in_maps = make_in_maps(x, w_qkv)
    res = run_bass_kernel_spmd(_CACHED_NC, in_maps, core_ids=list(range(NCORES)))
    out2 = assemble_output(res.results, x)
    return (x, out2)
